# revision 42
# baseline (speedup 1.0000x reference)
"""BiMambaTextEncoder Trainium2 kernel.

Sharding: 8 cores = 4 batch x 2 direction. The backward direction is handled
by reversing the token sequence on the host and flipping the conv kernels, so
all cores run the same SPMD program. The final projection decomposes as
  concat([fo, bo]) @ proj_w.T = fo @ proj_w[:, :C].T + bo @ proj_w[:, C:].T
so each core computes its half and the host sums the pair (no collectives).

Scan phase layout: for each (i-block, n-half) the 8 per-state scans are packed
into one [128, 8*513] tensor_tensor_scan; column n*513 is a boundary column
with dA=0 and dBu=carry-in state, which restarts the recurrence per state.
B/C broadcasts are done by DMA from a DRAM bounce buffer (0-stride partition
reads), the h*C multiplies run on the Pool engine, dt softplus and the LN
rsqrt are fused ACT ops, and PSUM->SBUF staging copies run on the ACT engine
to keep the Vector engine (the bottleneck) on scan + dBu work only.
"""

from contextlib import ExitStack

import numpy as np

import concourse.bass as bass
from concourse import bacc
import concourse.mybir as mybir
import concourse.tile as tile
from concourse.bass_utils import run_bass_kernel_spmd

F16 = mybir.dt.float16
F32 = mybir.dt.float32
AF = mybir.ActivationFunctionType
OP = mybir.AluOpType

B, L, C, K, DEPTH, V = 4, 1024, 512, 5, 3, 178
VP = 192            # padded vocab (two K-tiles: 128 + 64)
DI = 1024           # d_inner
N = 16              # d_state
DCONV = 4
DTR = 32            # dt_rank
NCB = C // 128      # 4 channel blocks
NDB = DI // 128     # 8 d_inner blocks
TC = 2              # t chunks of 512
Q = 512
EPS = 1e-5

NH = 2              # n halves per block
NPH = N // NH       # 8 states per packed scan
WB = NPH * (Q + 1)  # 4104 packed scan width (513-stride blocks)


def _par(param, cob):
    s = param * NCB + cob
    return slice(s, s + 1)


def build_program():
    nc = bacc.Bacc()

    d_h0 = nc.dram_tensor("h0", [C, L], F16, kind="ExternalInput")
    d_convw = nc.dram_tensor("convw", [DEPTH, NCB, 128, K * NCB, 128], F16,
                             kind="ExternalInput")
    d_cpar = nc.dram_tensor("cpar", [DEPTH, 128, 12], F32, kind="ExternalInput")
    d_inw = nc.dram_tensor("inw", [NCB, 128, 2 * DI], F16, kind="ExternalInput")
    d_mcw = nc.dram_tensor("mcw", [NDB, 128, DCONV * 128], F16,
                           kind="ExternalInput")
    d_mpar = nc.dram_tensor("mpar", [128, 16], F32, kind="ExternalInput")
    d_xw = nc.dram_tensor("xw", [NDB, 128, DTR + 2 * N], F16,
                          kind="ExternalInput")
    d_dtw = nc.dram_tensor("dtw", [NDB, DTR, 128], F16, kind="ExternalInput")
    d_An = nc.dram_tensor("An", [NDB, 128, N], F32, kind="ExternalInput")
    d_Dd = nc.dram_tensor("Dd", [NDB, 128, 128], F16, kind="ExternalInput")
    d_outw = nc.dram_tensor("outw", [NDB, 128, C], F16, kind="ExternalInput")
    d_pw = nc.dram_tensor("pw", [NCB, 128, C], F16, kind="ExternalInput")
    d_ident = nc.dram_tensor("ident", [128, 128], F16, kind="ExternalInput")
    d_part = nc.dram_tensor("part", [C, L], F32, kind="ExternalOutput")
    # DRAM bounce for B/C rows of x_proj output (for broadcast reads)
    d_xbc = nc.dram_tensor("xbc", [2 * N, L], F16)

    with tile.TileContext(nc) as tc, ExitStack() as ctx:
        sing = ctx.enter_context(tc.tile_pool(name="sing", bufs=1))
        wp = ctx.enter_context(tc.tile_pool(name="wp", bufs=1))
        hp = ctx.enter_context(tc.tile_pool(name="hp", bufs=1))
        bcp = ctx.enter_context(tc.tile_pool(name="bcp", bufs=1))
        sp = ctx.enter_context(tc.tile_pool(name="sp", bufs=1))
        st = ctx.enter_context(tc.tile_pool(name="st", bufs=1))
        pp = ctx.enter_context(tc.tile_pool(name="pp", bufs=1, space="PSUM"))

        dma = nc.sync.dma_start

        def T(pool, shape, dt, tag, bufs, name):
            return pool.tile(shape, dt, tag=tag, bufs=bufs, name=name)

        def r3(t):
            return t[:].rearrange("p (n q) -> p n q", n=NPH)

        # ---- constants / params ----
        ident = T(sing, [128, 128], F16, "ident", 1, "ident")
        dma(out=ident[:], in_=d_ident[:])
        ones = T(sing, [128, 1], F16, "ones", 1, "ones")
        nc.vector.memset(ones[:], 1.0)
        ones32 = T(sing, [128, 1], F32, "ones32", 1, "ones32")
        nc.vector.memset(ones32[:], 1.0)
        epst = T(sing, [1, 1], F32, "epst", 1, "epst")
        nc.vector.memset(epst[:], EPS)
        zcol = T(sing, [128, NPH], F16, "zcol", 1, "zcol")
        nc.vector.memset(zcol[:], 0.0)
        cpar = []
        for l in range(DEPTH):
            t = T(sing, [128, 12], F32, f"cpar{l}", 1, f"cpar{l}")
            dma(out=t[:], in_=d_cpar[l])
            cpar.append(t)
        mpar = T(sing, [128, 16], F32, "mpar", 1, "mpar")
        dma(out=mpar[:], in_=d_mpar[:])
        An = []
        for i in range(NDB):
            t = T(sing, [128, N], F32, f"An{i}", 1, f"An{i}")
            dma(out=t[:], in_=d_An[i])
            An.append(t)
        states = []
        for i in range(NDB):
            t = T(sing, [128, N], F16, f"stt{i}", 1, f"stt{i}")
            states.append(t)

        # pre-touch every ACT-consumed param tile on the scalar engine so the
        # real consumers don't exceed the Activation ISA sync-wait limit (the
        # engine's vector clock subsumes the DMA deps after one wait)
        touch = T(sing, [128, 224], F32, "touch", 1, "touch")
        for ti_, tt_ in enumerate(cpar + [mpar] + An):
            w_ = tt_.shape[-1]
            nc.scalar.copy(out=touch[:, ti_ * 16: ti_ * 16 + w_], in_=tt_[:])
        nc.scalar.copy(out=touch[0:1, 223:224], in_=epst[:])

        LP = L + 4
        hbuf = [[T(hp, [128, LP], F16, "big", 8, f"hbuf{s}_{cb}")
                 for cb in range(NCB)] for s in range(2)]
        for s in range(2):
            for cb in range(NCB):
                nc.vector.memset(hbuf[s][cb][:, 0:2], 0.0)
                nc.vector.memset(hbuf[s][cb][:, L + 2:LP], 0.0)

        for cb in range(NCB):
            dma(out=hbuf[0][cb][:, 2:2 + L],
                in_=d_h0[cb * 128:(cb + 1) * 128, :])

        # ---- conv stack ----
        for l in range(DEPTH):
            src = hbuf[l % 2]
            dst = hbuf[(l + 1) % 2]
            cw = []
            for cib in range(NCB):
                t = T(wp, [128, K * NCB * 128], F16, "cw", 4, f"cw{l}_{cib}")
                dma(out=t[:], in_=d_convw[l, cib])
                cw.append(t)
            craw = [T(st, [128, L], F16, "craw", 4, f"craw{l}_{cob}")
                    for cob in range(NCB)]
            for tq in range(TC):
                for cob in range(NCB):
                    ps = T(pp, [128, Q], F32, "mm", 2, f"ps_c{l}_{cob}_{tq}")
                    first = True
                    for cib in range(NCB):
                        for k in range(K):
                            j = k * NCB + cob
                            nc.tensor.matmul(
                                ps[:], cw[cib][:, j * 128:(j + 1) * 128],
                                src[cib][:, tq * Q + k: tq * Q + k + Q],
                                start=first,
                                stop=(cib == NCB - 1 and k == K - 1))
                            first = False
                    nc.scalar.activation(
                        out=craw[cob][:, tq * Q:(tq + 1) * Q], in_=ps[:],
                        func=AF.Identity, bias=cpar[l][:, _par(0, cob)],
                        scale=1.0)
                ps_s = T(pp, [1, Q], F32, "st", 2, f"ps_s{l}_{tq}")
                ps_q = T(pp, [1, Q], F32, "st", 2, f"ps_q{l}_{tq}")
                for cob in range(NCB):
                    nc.tensor.matmul(ps_s[:], ones[:],
                                     craw[cob][:, tq * Q:(tq + 1) * Q],
                                     start=(cob == 0), stop=(cob == NCB - 1))
                for cob in range(NCB):
                    sq = T(st, [128, Q], F16, "csq", 1, f"csq{l}_{cob}_{tq}")
                    nc.scalar.activation(out=sq[:],
                                         in_=craw[cob][:, tq * Q:(tq + 1) * Q],
                                         func=AF.Square)
                    nc.tensor.matmul(ps_q[:], ones[:], sq[:],
                                     start=(cob == 0), stop=(cob == NCB - 1))
                mu = T(st, [1, Q], F32, "row", 3, f"mu{l}_{tq}")
                nc.vector.tensor_scalar_mul(mu[:], ps_s[:], 1.0 / C)
                var = T(st, [1, Q], F32, "row", 3, f"var{l}_{tq}")
                # var = msq - mu^2
                nc.vector.tensor_mul(var[:], mu[:], mu[:])
                nc.vector.tensor_scalar_mul(var[:], var[:], -1.0)
                nc.vector.scalar_tensor_tensor(
                    out=var[:], in0=ps_q[:], scalar=1.0 / C, in1=var[:],
                    op0=OP.mult, op1=OP.add)
                nc.scalar.activation(out=var[:], in_=var[:], func=AF.Sqrt,
                                     bias=epst[:], scale=1.0)
                rstd = T(st, [1, Q], F32, "row", 3, f"rstd{l}_{tq}")
                nc.vector.reciprocal_approx_fast(out=rstd[:], in_=var[:])
                nmr = T(st, [1, Q], F16, "row16", 2, f"nmr{l}_{tq}")
                nc.vector.tensor_mul(nmr[:], mu[:], rstd[:])
                nc.vector.tensor_scalar_mul(nmr[:], nmr[:], -1.0)
                rstd16 = T(st, [1, Q], F16, "row16", 2, f"rstd16{l}_{tq}")
                nc.vector.tensor_copy(out=rstd16[:], in_=rstd[:])
                rs_bc = T(st, [128, Q], F16, "rs_bc", 1, f"rs_bc{l}_{tq}")
                nc.gpsimd.partition_broadcast(rs_bc[:], rstd16[:])
                nm_bc = T(st, [128, Q], F16, "nm_bc", 1, f"nm_bc{l}_{tq}")
                nc.gpsimd.partition_broadcast(nm_bc[:], nmr[:])
                for cob in range(NCB):
                    t2 = T(st, [128, Q], F16, "lnt", 1, f"lnt{l}_{cob}_{tq}")
                    nc.vector.tensor_mul(t2[:],
                                         craw[cob][:, tq * Q:(tq + 1) * Q],
                                         rs_bc[:])
                    nc.vector.tensor_add(t2[:], t2[:], nm_bc[:])
                    nc.scalar.activation(
                        out=dst[cob][:, 2 + tq * Q: 2 + (tq + 1) * Q],
                        in_=t2[:], func=AF.Prelu,
                        bias=cpar[l][:, _par(2, cob)],
                        scale=cpar[l][:, _par(1, cob)], alpha=0.2)

        hfin = hbuf[DEPTH % 2]

        # ---- in_proj ----
        inw = []
        for cib in range(NCB):
            t = T(wp, [128, 2 * DI], F16, "cw", 4, f"inw{cib}")
            dma(out=t[:], in_=d_inw[cib])
            inw.append(t)
        LPAD = 3
        ubuf = [T(hp, [128, L + LPAD], F16, "mid", 8, f"ubuf{i}")
                for i in range(NDB)]
        for i in range(NDB):
            nc.vector.memset(ubuf[i][:, 0:LPAD], 0.0)
        silz = [T(hp, [128, L], F16, f"silz{i}", 1, f"silz{i}")
                for i in range(NDB)]
        def in_proj_half(ms, ts):
            for m in ms:
                for t in ts:
                    ps = T(pp, [128, Q], F32, "mm", 2, f"ps_in{m}_{t}")
                    for cib in range(NCB):
                        nc.tensor.matmul(
                            ps[:], inw[cib][:, m * 128:(m + 1) * 128],
                            hfin[cib][:, 2 + t * Q: 2 + (t + 1) * Q],
                            start=(cib == 0), stop=(cib == NCB - 1))
                    if m < NDB:
                        # DVE idle during in_proj; keep ACT free for silus
                        nc.vector.tensor_copy(
                            out=ubuf[m][:, LPAD + t * Q: LPAD + (t + 1) * Q],
                            in_=ps[:])
                    else:
                        nc.scalar.activation(
                            out=silz[m - NDB][:, t * Q:(t + 1) * Q],
                            in_=ps[:], func=AF.Silu)

        uconv = [T(hp, [128, L], F16, f"uconv{i}", 1, f"uconv{i}")
                 for i in range(NDB)]

        def mamba_conv_t(t):
            for i in range(NDB):
                mcwt = T(wp, [128, DCONV * 128], F16, "mcw", 1,
                         f"mcw{t}_{i}")
                dma(out=mcwt[:], in_=d_mcw[i])
                ps = T(pp, [128, Q], F32, "mm", 2, f"ps_mc{i}_{t}")
                for k in range(DCONV):
                    nc.tensor.matmul(
                        ps[:], mcwt[:, k * 128:(k + 1) * 128],
                        ubuf[i][:, t * Q + k: t * Q + k + Q],
                        start=(k == 0), stop=(k == DCONV - 1))
                nc.scalar.activation(
                    out=uconv[i][:, t * Q:(t + 1) * Q], in_=ps[:],
                    func=AF.Silu, bias=mpar[:, i:i + 1], scale=1.0)

        xw = []
        for i in range(NDB):
            t = T(wp, [128, DTR + 2 * N], F16, f"xw{i}", 1, f"xw{i}")
            dma(out=t[:], in_=d_xw[i])
            xw.append(t)
        xdbc = T(hp, [DTR + 2 * N, L], F16, "xdbc", 1, "xdbc")

        def x_proj_t(t):
            ps = T(pp, [DTR + 2 * N, Q], F32, "mm", 2, f"ps_x{t}")
            for i in range(NDB):
                nc.tensor.matmul(ps[:], xw[i][:],
                                 uconv[i][:, t * Q:(t + 1) * Q],
                                 start=(i == 0), stop=(i == NDB - 1))
            nc.vector.tensor_copy(out=xdbc[:, t * Q:(t + 1) * Q], in_=ps[:])
            dma(out=d_xbc[:, t * Q:(t + 1) * Q],
                in_=xdbc[DTR:DTR + 2 * N, t * Q:(t + 1) * Q])

        # the chain to the first scan needs only the t=0 chunk; everything
        # else is emitted at rock-bottom priority so the scheduler treats it
        # as fill work during the scan phase
        in_proj_half(range(NDB), [0])
        mamba_conv_t(0)
        x_proj_t(0)
        with tc.high_priority(offset=-1000000):
            in_proj_half(range(NDB, 2 * NDB), range(TC))
            in_proj_half(range(NDB), [1])
            mamba_conv_t(1)
            x_proj_t(1)

        # ---- small weights ----
        dtw = []
        for i in range(NDB):
            t = T(wp, [DTR, 128], F16, f"dtw{i}", 1, f"dtw{i}")
            dma(out=t[:], in_=d_dtw[i])
            dtw.append(t)
        Dd = []
        for i in range(NDB):
            t = T(wp, [128, 128], F16, f"Dd{i}", 1, f"Dd{i}")
            dma(out=t[:], in_=d_Dd[i])
            Dd.append(t)

        # ---- scan phase (t-chunked; packed 8-state scans; state chained) ----
        yfin = [T(hp, [128, L], F16, "big", 8, f"yfin{i}")
                for i in range(NDB)]
        for tq in range(TC):
            sl = slice(tq * Q, (tq + 1) * Q)
            # B/C broadcast tiles per half via DMA (0-stride partition reads)
            ball, call_ = [], []
            for h in range(NH):
                bt = T(bcp, [128, WB], F16, "ball", 2, f"ball{tq}_{h}")
                for p0 in (0, 64):
                    dma(out=r3(bt)[p0:p0 + 64, :, 1:Q + 1],
                        in_=d_xbc[NPH * h:NPH * h + NPH,
                                  sl].partition_broadcast(64))
                ball.append(bt)
                ct = T(bcp, [128, WB], F16, "call", 2, f"call{tq}_{h}")
                for p0 in (0, 64):
                    dma(out=r3(ct)[p0:p0 + 64, :, 1:Q + 1],
                        in_=d_xbc[N + NPH * h:N + NPH * h + NPH,
                                  sl].partition_broadcast(64))
                call_.append(ct)
            # dt_proj -> softplus -> delta; du = delta * u
            deltas = []
            dus = []
            for i in range(NDB):
                ps = T(pp, [128, Q], F32, "mm", 2, f"ps_dt{tq}_{i}")
                nc.tensor.matmul(ps[:], dtw[i][:], xdbc[0:DTR, sl],
                                 start=True, stop=True)
                dl = T(sp, [128, Q], F16, "delta", 8, f"delta{tq}_{i}")
                nc.scalar.activation(out=dl[:], in_=ps[:], func=AF.Exp,
                                     bias=mpar[:, 8 + i:9 + i], scale=1.0)
                deltas.append(dl)
            for i in range(NDB):
                # softplus tail in place: ln(exp(x) + 1), +1 fused as bias
                nc.scalar.activation(out=deltas[i][:], in_=deltas[i][:],
                                     func=AF.Ln, bias=ones32[:], scale=1.0)
            pend = []
            for i in range(NDB):
                yp = T(pp, [128, Q], F32, "y", 2, f"yp{tq}_{i}")
                nc.tensor.matmul(yp[:], Dd[i][:], uconv[i][:, sl],
                                 start=True, stop=False)
                du = T(sp, [128, Q], F16, "du", 2, f"du{tq}_{i}")
                nc.vector.tensor_mul(du[:], deltas[i][:], uconv[i][:, sl])
                dus.append(du)
                # deferred yfin of the previous block so the Vector engine
                # never waits on the Pool/PE tail of iteration i-1
                if pend:
                    pi, pyp = pend.pop()
                    yq = T(sp, [128, Q], F16, "du", 2, f"yq{tq}_{pi}")
                    nc.scalar.copy(out=yq[:], in_=pyp[:])
                    nc.vector.tensor_mul(yfin[pi][:, sl], yq[:],
                                         silz[pi][:, sl])
                for h in range(NH):
                    da = T(sp, [128, WB], F16, "DA", 2, f"da{tq}_{i}_{h}")
                    da3 = r3(da)
                    if tq == 0 and i == 0:
                        # boundary zeros persist across buffer reuse: the
                        # exps only ever write the 513-stride interiors, so
                        # zeroing the first two allocations (both bufs) is
                        # enough for every later tile in the cycle
                        nc.vector.tensor_copy(out=da3[:, :, 0:1],
                                              in_=zcol[:])
                    for nl in range(NPH):
                        n = h * NPH + nl
                        nc.scalar.activation(
                            out=da3[:, nl:nl + 1, 1:Q + 1],
                            in_=deltas[i][:], func=AF.Exp,
                            scale=An[i][:, n:n + 1])
                    dbu = T(sp, [128, WB], F16, "DBU", 1, f"dbu{tq}_{i}_{h}")
                    dbu3 = r3(dbu)
                    if tq == 0:
                        # single buffer: zeros written once keep holding
                        if i == 0 and h == 0:
                            nc.vector.tensor_copy(out=dbu3[:, :, 0:1],
                                                  in_=zcol[:])
                    else:
                        nc.vector.tensor_copy(
                            out=dbu3[:, :, 0:1],
                            in_=states[i][:, h * NPH:(h + 1) * NPH])
                    du_b = dus[i][:].unsqueeze(1).broadcast_to(
                        [128, NPH, Q])
                    nc.vector.tensor_tensor(out=dbu3[:, :, 1:Q + 1],
                                            in0=du_b,
                                            in1=r3(ball[h])[:, :, 1:Q + 1],
                                            op=OP.mult)
                    ht = T(sp, [128, WB], F16, "H", 2, f"h{tq}_{i}_{h}")
                    nc.vector.tensor_tensor_scan(ht[:], da[:], dbu[:], 0.0,
                                                 OP.mult, OP.add)
                    h3 = r3(ht)
                    if tq < TC - 1:
                        nc.vector.tensor_copy(
                            out=states[i][:, h * NPH:(h + 1) * NPH],
                            in_=h3[:, :, Q:Q + 1])
                    hc = T(sp, [128, NPH * Q], F16, "HC", 1,
                           f"hc{tq}_{i}_{h}")
                    hc3 = hc[:].rearrange("p (n q) -> p n q", n=NPH)
                    nc.vector.tensor_tensor(out=hc3, in0=h3[:, :, 1:Q + 1],
                                            in1=r3(call_[h])[:, :, 1:Q + 1],
                                            op=OP.mult)
                    for nl in range(NPH):
                        nc.tensor.matmul(yp[:], ident[:],
                                         hc[:, nl * Q:(nl + 1) * Q],
                                         start=False,
                                         stop=(h == NH - 1 and nl == NPH - 1))
                pend.append((i, yp))
            pi, pyp = pend.pop()
            yq = T(sp, [128, Q], F16, "du", 2, f"yq{tq}_{pi}")
            nc.scalar.copy(out=yq[:], in_=pyp[:])
            nc.vector.tensor_mul(yfin[pi][:, sl], yq[:], silz[pi][:, sl])

        # ---- out_proj + final proj, emitted per t-chunk ----
        yo = [T(hp, [128, L], F16, "mid", 8, f"yo{cb}") for cb in range(NCB)]
        pw = []

        def tail_chunk(t):
            # out_proj: outw streamed through the (now free) craw-tag bufs
            # in two waves of 4 per output block
            for cb in range(NCB):
                ps = T(pp, [128, Q], F32, "dtm", 2, f"ps_o{cb}_{t}")
                for wv in range(2):
                    oww = []
                    for j in range(4):
                        i = wv * 4 + j
                        w_ = T(st, [128, C], F16, "craw", 4,
                               f"outw{t}_{cb}_{i}")
                        dma(out=w_[:], in_=d_outw[i])
                        oww.append(w_)
                    for j in range(4):
                        i = wv * 4 + j
                        nc.tensor.matmul(
                            ps[:], oww[j][:, cb * 128:(cb + 1) * 128],
                            yfin[i][:, t * Q:(t + 1) * Q],
                            start=(i == 0), stop=(i == NDB - 1))
                nc.vector.tensor_copy(out=yo[cb][:, t * Q:(t + 1) * Q],
                                      in_=ps[:])
            if not pw:
                for cib in range(NCB):
                    w_ = T(hp, [128, C], F16, f"uconv{cib}", 1, f"pw{cib}")
                    dma(out=w_[:], in_=d_pw[cib])
                    pw.append(w_)
            for cb in range(NCB):
                ps = T(pp, [128, Q], F32, "mm", 2, f"ps_p{cb}_{t}")
                for cib in range(NCB):
                    nc.tensor.matmul(ps[:],
                                     pw[cib][:, cb * 128:(cb + 1) * 128],
                                     yo[cib][:, t * Q:(t + 1) * Q],
                                     start=(cib == 0), stop=(cib == NCB - 1))
                ot = T(hp, [128, Q], F32, "osb", 1, f"osb{cb}_{t}")
                nc.scalar.copy(out=ot[:], in_=ps[:])
                dma(out=d_part[cb * 128:(cb + 1) * 128, t * Q:(t + 1) * Q],
                    in_=ot[:])

        with tc.high_priority(offset=-1000000):
            tail_chunk(0)
        tail_chunk(1)

    nc.compile()
    return nc


_cache = {}


def _prep_core_inputs(inputs, core):
    b = core >> 1
    rev = (core & 1) == 1
    p = "b_" if rev else "f_"
    f16 = np.float16
    f32 = np.float32

    toks = np.asarray(inputs["x"][b]).astype(np.int64)
    if rev:
        toks = toks[::-1]
    embf = np.asarray(inputs["emb"]).astype(f16)
    h0 = np.ascontiguousarray(embf[toks].T)

    key = ("wts", p)
    if key not in _cache:

        cw = np.asarray(inputs["conv_w"]).astype(f32)  # [D, cout, cin, K]
        if rev:
            cw = cw[:, :, :, ::-1]
        convw = np.empty((DEPTH, NCB, 128, K * NCB, 128), f16)
        for l in range(DEPTH):
            for cib in range(NCB):
                for k in range(K):
                    for cob in range(NCB):
                        blk = cw[l, cob * 128:(cob + 1) * 128,
                                 cib * 128:(cib + 1) * 128, k]
                        convw[l, cib, :, k * NCB + cob, :] = blk.T.astype(f16)
        cpar = np.zeros((DEPTH, 128, 12), f32)
        for l in range(DEPTH):
            for cob in range(NCB):
                cs = slice(cob * 128, (cob + 1) * 128)
                cpar[l, :, 0 * NCB + cob] = inputs["conv_b"][l][cs]
                cpar[l, :, 1 * NCB + cob] = inputs["ln_g"][l][cs]
                cpar[l, :, 2 * NCB + cob] = inputs["ln_b"][l][cs]

        in_w = np.asarray(inputs[p + "in_w"]).astype(f32)  # [2*DI, C]
        inw = np.empty((NCB, 128, 2 * DI), f16)
        for cib in range(NCB):
            inw[cib] = in_w[:, cib * 128:(cib + 1) * 128].T.astype(f16)

        mconv = np.asarray(inputs[p + "conv_w"]).astype(f32)  # [DI, 4]
        mcw = np.zeros((NDB, 128, DCONV * 128), f16)
        dd = np.arange(128)
        for i in range(NDB):
            for k in range(DCONV):
                mcw[i, dd, k * 128 + dd] = mconv[i * 128:(i + 1) * 128, k]

        mpar = np.zeros((128, 16), f32)
        for i in range(NDB):
            mpar[:, i] = inputs[p + "conv_b"][i * 128:(i + 1) * 128]
            mpar[:, 8 + i] = inputs[p + "dt_b"][i * 128:(i + 1) * 128]

        x_w = np.asarray(inputs[p + "x_w"]).astype(f32)  # [64, DI]
        xw = np.empty((NDB, 128, DTR + 2 * N), f16)
        for i in range(NDB):
            xw[i] = x_w[:, i * 128:(i + 1) * 128].T.astype(f16)

        dt_w = np.asarray(inputs[p + "dt_w"]).astype(f32)  # [DI, DTR]
        dtw = np.empty((NDB, DTR, 128), f16)
        for i in range(NDB):
            dtw[i] = dt_w[i * 128:(i + 1) * 128, :].T.astype(f16)

        An = (-np.exp(np.asarray(inputs[p + "A_log"]).astype(f32))
              ).reshape(NDB, 128, N).astype(f32)

        Dv = np.asarray(inputs[p + "D"]).astype(f32)
        Dd = np.zeros((NDB, 128, 128), f16)
        for i in range(NDB):
            Dd[i, dd, dd] = Dv[i * 128:(i + 1) * 128]

        out_w = np.asarray(inputs[p + "out_w"]).astype(f32)  # [C, DI]
        outw = np.empty((NDB, 128, C), f16)
        for i in range(NDB):
            outw[i] = out_w[:, i * 128:(i + 1) * 128].T.astype(f16)

        proj_w = np.asarray(inputs["proj_w"]).astype(f32)  # [C, 2C]
        half = proj_w[:, C:] if rev else proj_w[:, :C]
        pw = np.empty((NCB, 128, C), f16)
        for cib in range(NCB):
            pw[cib] = half[:, cib * 128:(cib + 1) * 128].T.astype(f16)

        _cache[key] = dict(
            convw=convw, cpar=cpar, inw=inw, mcw=mcw, mpar=mpar,
            xw=xw, dtw=dtw, An=An, Dd=Dd, outw=outw, pw=pw,
            ident=np.eye(128, dtype=f16))
    m = dict(_cache[key])
    m["h0"] = h0
    return m


def kernel(**inputs):
    if "nc" not in _cache:
        _cache["nc"] = build_program()
    nc = _cache["nc"]
    # weights are cached per direction for repeat calls; invalidate so a new
    # inputs dict is always re-prepared
    for k in [k for k in _cache if k != "nc"]:
        del _cache[k]
    in_maps = [_prep_core_inputs(inputs, c) for c in range(8)]
    res = run_bass_kernel_spmd(nc, in_maps, list(range(8)))
    parts = [r["part"] for r in res.results]
    proj_b = np.asarray(inputs["proj_b"]).astype(np.float32)
    out = np.empty((B, L, C), np.float32)
    for b in range(B):
        # note: the reference concatenates bo still in reversed time order
        comb = parts[2 * b] + parts[2 * b + 1]
        out[b] = comb.T + proj_b[None, :]
    m = np.asarray(inputs["m"])
    out = np.where(m[:, :, None], 0.0, out).astype(np.float32)
    return out


# revision 45
# speedup vs baseline: 1.0692x; 1.0692x over previous
"""BiMambaTextEncoder Trainium2 kernel.

Sharding: 8 cores = 4 batch x 2 direction. The backward direction is handled
by reversing the token sequence on the host and flipping the conv kernels, so
all cores run the same SPMD program. The final projection decomposes as
  concat([fo, bo]) @ proj_w.T = fo @ proj_w[:, :C].T + bo @ proj_w[:, C:].T
so each core computes its half and the host sums the pair (no collectives).

Scan phase layout: for each (i-block, n-half) the 8 per-state scans are packed
into one [128, 8*513] tensor_tensor_scan; column n*513 is a boundary column
with dA=0 and dBu=carry-in state, which restarts the recurrence per state.
B/C broadcasts are done by DMA from a DRAM bounce buffer (0-stride partition
reads), the h*C multiplies run on the Pool engine, dt softplus and the LN
rsqrt are fused ACT ops, and PSUM->SBUF staging copies run on the ACT engine
to keep the Vector engine (the bottleneck) on scan + dBu work only.
"""

from contextlib import ExitStack

import numpy as np

import concourse.bass as bass
from concourse import bacc
import concourse.mybir as mybir
import concourse.tile as tile
from concourse.bass_utils import run_bass_kernel_spmd

F16 = mybir.dt.float16
F32 = mybir.dt.float32
AF = mybir.ActivationFunctionType
OP = mybir.AluOpType

B, L, C, K, DEPTH, V = 4, 1024, 512, 5, 3, 178
VP = 192            # padded vocab (two K-tiles: 128 + 64)
DI = 1024           # d_inner
N = 16              # d_state
DCONV = 4
DTR = 32            # dt_rank
NCB = C // 128      # 4 channel blocks
NDB = DI // 128     # 8 d_inner blocks
TC = 2              # t chunks of 512
Q = 512
EPS = 1e-5

NH = 2              # n halves per block
NPH = N // NH       # 8 states per packed scan
WB = NPH * (Q + 1)  # 4104 packed scan width (513-stride blocks)


def _par(param, cob):
    s = param * NCB + cob
    return slice(s, s + 1)


def build_program():
    nc = bacc.Bacc()

    d_h0 = nc.dram_tensor("h0", [C, L], F16, kind="ExternalInput")
    d_convw = nc.dram_tensor("convw", [DEPTH, NCB, 128, K * NCB, 128], F16,
                             kind="ExternalInput")
    d_cpar = nc.dram_tensor("cpar", [DEPTH, 128, 12], F32, kind="ExternalInput")
    d_inw = nc.dram_tensor("inw", [NCB, 128, 2 * DI], F16, kind="ExternalInput")
    d_mcw = nc.dram_tensor("mcw", [NDB, 128, DCONV * 128], F16,
                           kind="ExternalInput")
    d_mpar = nc.dram_tensor("mpar", [128, 16], F32, kind="ExternalInput")
    d_xw = nc.dram_tensor("xw", [NDB, 128, DTR + 2 * N], F16,
                          kind="ExternalInput")
    d_dtw = nc.dram_tensor("dtw", [NDB, DTR, 128], F16, kind="ExternalInput")
    d_An = nc.dram_tensor("An", [NDB, 128, N], F32, kind="ExternalInput")
    d_Dd = nc.dram_tensor("Dd", [NDB, 128, 128], F16, kind="ExternalInput")
    d_outw = nc.dram_tensor("outw", [NDB, 128, C], F16, kind="ExternalInput")
    d_pw = nc.dram_tensor("pw", [NCB, 128, C], F16, kind="ExternalInput")
    d_ident = nc.dram_tensor("ident", [128, 128], F16, kind="ExternalInput")
    d_part = nc.dram_tensor("part", [C, L], F32, kind="ExternalOutput")
    # DRAM bounce for B/C rows of x_proj output (for broadcast reads)
    d_xbc = nc.dram_tensor("xbc", [2 * N, L], F16)

    with tile.TileContext(nc) as tc, ExitStack() as ctx:
        sing = ctx.enter_context(tc.tile_pool(name="sing", bufs=1))
        wp = ctx.enter_context(tc.tile_pool(name="wp", bufs=1))
        hp = ctx.enter_context(tc.tile_pool(name="hp", bufs=1))
        bcp = ctx.enter_context(tc.tile_pool(name="bcp", bufs=1))
        sp = ctx.enter_context(tc.tile_pool(name="sp", bufs=1))
        st = ctx.enter_context(tc.tile_pool(name="st", bufs=1))
        pp = ctx.enter_context(tc.tile_pool(name="pp", bufs=1, space="PSUM"))

        dma = nc.sync.dma_start

        def T(pool, shape, dt, tag, bufs, name):
            return pool.tile(shape, dt, tag=tag, bufs=bufs, name=name)

        def r3(t):
            return t[:].rearrange("p (n q) -> p n q", n=NPH)

        # ---- constants / params ----
        ident = T(sing, [128, 128], F16, "ident", 1, "ident")
        dma(out=ident[:], in_=d_ident[:])
        ones = T(sing, [128, 1], F16, "ones", 1, "ones")
        nc.vector.memset(ones[:], 1.0)
        ones32 = T(sing, [128, 1], F32, "ones32", 1, "ones32")
        nc.vector.memset(ones32[:], 1.0)
        epst = T(sing, [1, 1], F32, "epst", 1, "epst")
        nc.vector.memset(epst[:], EPS)
        zcol = T(sing, [128, NPH], F16, "zcol", 1, "zcol")
        nc.vector.memset(zcol[:], 0.0)
        cpar = []
        for l in range(DEPTH):
            t = T(sing, [128, 12], F32, f"cpar{l}", 1, f"cpar{l}")
            dma(out=t[:], in_=d_cpar[l])
            cpar.append(t)
        mpar = T(sing, [128, 16], F32, "mpar", 1, "mpar")
        dma(out=mpar[:], in_=d_mpar[:])
        An = []
        for i in range(NDB):
            t = T(sing, [128, N], F32, f"An{i}", 1, f"An{i}")
            dma(out=t[:], in_=d_An[i])
            An.append(t)
        states = []
        for i in range(NDB):
            t = T(sing, [128, N], F16, f"stt{i}", 1, f"stt{i}")
            states.append(t)

        # pre-touch every ACT-consumed param tile on the scalar engine so the
        # real consumers don't exceed the Activation ISA sync-wait limit (the
        # engine's vector clock subsumes the DMA deps after one wait)
        touch = T(sing, [128, 224], F16, "touch", 1, "touch")
        for ti_, tt_ in enumerate(cpar + [mpar] + An):
            w_ = tt_.shape[-1]
            nc.scalar.copy(out=touch[:, ti_ * 16: ti_ * 16 + w_], in_=tt_[:])
        nc.scalar.copy(out=touch[0:1, 223:224], in_=epst[:])

        LP = L + 4
        hbuf = [[T(hp, [128, LP], F16, "big", 8, f"hbuf{s}_{cb}")
                 for cb in range(NCB)] for s in range(2)]
        for s in range(2):
            for cb in range(NCB):
                nc.vector.memset(hbuf[s][cb][:, 0:2], 0.0)
                nc.vector.memset(hbuf[s][cb][:, L + 2:LP], 0.0)

        for cb in range(NCB):
            dma(out=hbuf[0][cb][:, 2:2 + L],
                in_=d_h0[cb * 128:(cb + 1) * 128, :])

        # ---- conv stack ----
        for l in range(DEPTH):
            src = hbuf[l % 2]
            dst = hbuf[(l + 1) % 2]
            cw = []
            for cib in range(NCB):
                t = T(wp, [128, K * NCB * 128], F16, "cw", 4, f"cw{l}_{cib}")
                dma(out=t[:], in_=d_convw[l, cib])
                cw.append(t)
            craw = [T(st, [128, L], F16, "craw", 4, f"craw{l}_{cob}")
                    for cob in range(NCB)]
            for tq in range(TC):
                for cob in range(NCB):
                    ps = T(pp, [128, Q], F32, "mm", 2, f"ps_c{l}_{cob}_{tq}")
                    first = True
                    for cib in range(NCB):
                        for k in range(K):
                            j = k * NCB + cob
                            nc.tensor.matmul(
                                ps[:], cw[cib][:, j * 128:(j + 1) * 128],
                                src[cib][:, tq * Q + k: tq * Q + k + Q],
                                start=first,
                                stop=(cib == NCB - 1 and k == K - 1))
                            first = False
                    nc.scalar.activation(
                        out=craw[cob][:, tq * Q:(tq + 1) * Q], in_=ps[:],
                        func=AF.Identity, bias=cpar[l][:, _par(0, cob)],
                        scale=1.0)
                ps_s = T(pp, [1, Q], F32, "st", 2, f"ps_s{l}_{tq}")
                ps_q = T(pp, [1, Q], F32, "st", 2, f"ps_q{l}_{tq}")
                for cob in range(NCB):
                    nc.tensor.matmul(ps_s[:], ones[:],
                                     craw[cob][:, tq * Q:(tq + 1) * Q],
                                     start=(cob == 0), stop=(cob == NCB - 1))
                for cob in range(NCB):
                    sq = T(st, [128, Q], F16, "csq", 1, f"csq{l}_{cob}_{tq}")
                    nc.scalar.activation(out=sq[:],
                                         in_=craw[cob][:, tq * Q:(tq + 1) * Q],
                                         func=AF.Square)
                    nc.tensor.matmul(ps_q[:], ones[:], sq[:],
                                     start=(cob == 0), stop=(cob == NCB - 1))
                mu = T(st, [1, Q], F32, "row", 3, f"mu{l}_{tq}")
                nc.vector.tensor_scalar_mul(mu[:], ps_s[:], 1.0 / C)
                var = T(st, [1, Q], F32, "row", 3, f"var{l}_{tq}")
                # var = msq - mu^2
                nc.vector.tensor_mul(var[:], mu[:], mu[:])
                nc.vector.tensor_scalar_mul(var[:], var[:], -1.0)
                nc.vector.scalar_tensor_tensor(
                    out=var[:], in0=ps_q[:], scalar=1.0 / C, in1=var[:],
                    op0=OP.mult, op1=OP.add)
                nc.scalar.activation(out=var[:], in_=var[:], func=AF.Sqrt,
                                     bias=epst[:], scale=1.0)
                rstd = T(st, [1, Q], F32, "row", 3, f"rstd{l}_{tq}")
                nc.vector.reciprocal_approx_fast(out=rstd[:], in_=var[:])
                nmr = T(st, [1, Q], F16, "row16", 2, f"nmr{l}_{tq}")
                nc.vector.tensor_mul(nmr[:], mu[:], rstd[:])
                nc.vector.tensor_scalar_mul(nmr[:], nmr[:], -1.0)
                rstd16 = T(st, [1, Q], F16, "row16", 2, f"rstd16{l}_{tq}")
                nc.vector.tensor_copy(out=rstd16[:], in_=rstd[:])
                rs_bc = T(st, [128, Q], F16, "rs_bc", 1, f"rs_bc{l}_{tq}")
                nc.gpsimd.partition_broadcast(rs_bc[:], rstd16[:])
                nm_bc = T(st, [128, Q], F16, "nm_bc", 1, f"nm_bc{l}_{tq}")
                nc.gpsimd.partition_broadcast(nm_bc[:], nmr[:])
                for cob in range(NCB):
                    t2 = T(st, [128, Q], F16, "lnt", 1, f"lnt{l}_{cob}_{tq}")
                    nc.vector.tensor_mul(t2[:],
                                         craw[cob][:, tq * Q:(tq + 1) * Q],
                                         rs_bc[:])
                    nc.vector.tensor_add(t2[:], t2[:], nm_bc[:])
                    nc.scalar.activation(
                        out=dst[cob][:, 2 + tq * Q: 2 + (tq + 1) * Q],
                        in_=t2[:], func=AF.Prelu,
                        bias=cpar[l][:, _par(2, cob)],
                        scale=cpar[l][:, _par(1, cob)], alpha=0.2)

        hfin = hbuf[DEPTH % 2]

        # ---- in_proj ----
        inw = []
        for cib in range(NCB):
            t = T(wp, [128, 2 * DI], F16, "cw", 4, f"inw{cib}")
            dma(out=t[:], in_=d_inw[cib])
            inw.append(t)
        LPAD = 3
        ubuf = [T(hp, [128, L + LPAD], F16, "mid", 8, f"ubuf{i}")
                for i in range(NDB)]
        for i in range(NDB):
            nc.vector.memset(ubuf[i][:, 0:LPAD], 0.0)
        silz = [T(hp, [128, L], F16, f"silz{i}", 1, f"silz{i}")
                for i in range(NDB)]
        def in_proj_half(ms):
            for m in ms:
                for t in range(TC):
                    ps = T(pp, [128, Q], F32, "mm", 2, f"ps_in{m}_{t}")
                    for cib in range(NCB):
                        nc.tensor.matmul(
                            ps[:], inw[cib][:, m * 128:(m + 1) * 128],
                            hfin[cib][:, 2 + t * Q: 2 + (t + 1) * Q],
                            start=(cib == 0), stop=(cib == NCB - 1))
                    if m < NDB:
                        # DVE idle during in_proj; keep ACT free for silus
                        nc.vector.tensor_copy(
                            out=ubuf[m][:, LPAD + t * Q: LPAD + (t + 1) * Q],
                            in_=ps[:])
                    else:
                        nc.scalar.activation(
                            out=silz[m - NDB][:, t * Q:(t + 1) * Q],
                            in_=ps[:], func=AF.Silu)

        # u half only: the z half is deferred past x_proj so the critical
        # chain to the first scan is not stalled behind its 64 matmuls
        in_proj_half(range(NDB))

        # ---- causal depthwise conv + SiLU ----
        mcw = []
        for i in range(NDB):
            t = T(wp, [128, DCONV * 128], F16, "mcw", 2, f"mcw{i}")
            dma(out=t[:], in_=d_mcw[i])
            mcw.append(t)
        uconv = [T(hp, [128, L], F16, f"uconv{i}", 1, f"uconv{i}")
                 for i in range(NDB)]
        for i in range(NDB):
            for t in range(TC):
                ps = T(pp, [128, Q], F32, "mm", 2, f"ps_mc{i}_{t}")
                for k in range(DCONV):
                    nc.tensor.matmul(
                        ps[:], mcw[i][:, k * 128:(k + 1) * 128],
                        ubuf[i][:, t * Q + k: t * Q + k + Q],
                        start=(k == 0), stop=(k == DCONV - 1))
                nc.scalar.activation(
                    out=uconv[i][:, t * Q:(t + 1) * Q], in_=ps[:],
                    func=AF.Silu, bias=mpar[:, i:i + 1], scale=1.0)

        # ---- x_proj ----
        xw = []
        for i in range(NDB):
            t = T(wp, [128, DTR + 2 * N], F16, f"xw{i}", 1, f"xw{i}")
            dma(out=t[:], in_=d_xw[i])
            xw.append(t)
        xdbc = T(hp, [DTR + 2 * N, L], F16, "xdbc", 1, "xdbc")
        for t in range(TC):
            ps = T(pp, [DTR + 2 * N, Q], F32, "mm", 2, f"ps_x{t}")
            for i in range(NDB):
                nc.tensor.matmul(ps[:], xw[i][:],
                                 uconv[i][:, t * Q:(t + 1) * Q],
                                 start=(i == 0), stop=(i == NDB - 1))
            nc.vector.tensor_copy(out=xdbc[:, t * Q:(t + 1) * Q], in_=ps[:])
        # dump B/C rows to DRAM for broadcast reads
        dma(out=d_xbc[:], in_=xdbc[DTR:DTR + 2 * N, :])
        # deferred z half of in_proj (only needed by the yfin gates)
        in_proj_half(range(NDB, 2 * NDB))

        # ---- small weights ----
        dtw = []
        for i in range(NDB):
            t = T(wp, [DTR, 128], F16, f"dtw{i}", 1, f"dtw{i}")
            dma(out=t[:], in_=d_dtw[i])
            dtw.append(t)
        Dd = []
        for i in range(NDB):
            t = T(wp, [128, 128], F16, f"Dd{i}", 1, f"Dd{i}")
            dma(out=t[:], in_=d_Dd[i])
            Dd.append(t)

        # ---- scan phase (t-chunked; packed 8-state scans; state chained) ----
        yfin = [T(hp, [128, L], F16, "big", 8, f"yfin{i}")
                for i in range(NDB)]
        for tq in range(TC):
            sl = slice(tq * Q, (tq + 1) * Q)
            # B/C broadcast tiles per half via DMA (0-stride partition reads)
            ball, call_ = [], []
            for h in range(NH):
                bt = T(bcp, [128, WB], F16, "ball", 2, f"ball{tq}_{h}")
                for p0 in (0, 64):
                    dma(out=r3(bt)[p0:p0 + 64, :, 1:Q + 1],
                        in_=d_xbc[NPH * h:NPH * h + NPH,
                                  sl].partition_broadcast(64))
                ball.append(bt)
                ct = T(bcp, [128, WB], F16, "call", 2, f"call{tq}_{h}")
                for p0 in (0, 64):
                    dma(out=r3(ct)[p0:p0 + 64, :, 1:Q + 1],
                        in_=d_xbc[N + NPH * h:N + NPH * h + NPH,
                                  sl].partition_broadcast(64))
                call_.append(ct)
            # dt_proj -> softplus -> delta; du = delta * u
            deltas = []
            dus = []
            for i in range(NDB):
                ps = T(pp, [128, Q], F32, "mm", 2, f"ps_dt{tq}_{i}")
                nc.tensor.matmul(ps[:], dtw[i][:], xdbc[0:DTR, sl],
                                 start=True, stop=True)
                dl = T(sp, [128, Q], F16, "delta", 8, f"delta{tq}_{i}")
                nc.scalar.activation(out=dl[:], in_=ps[:], func=AF.Exp,
                                     bias=mpar[:, 8 + i:9 + i], scale=1.0)
                deltas.append(dl)
            for i in range(NDB):
                # softplus tail in place: ln(exp(x) + 1), +1 fused as bias
                nc.scalar.activation(out=deltas[i][:], in_=deltas[i][:],
                                     func=AF.Ln, bias=ones32[:], scale=1.0)
            pend = []
            for i in range(NDB):
                yp = T(pp, [128, Q], F32, "y", 2, f"yp{tq}_{i}")
                nc.tensor.matmul(yp[:], Dd[i][:], uconv[i][:, sl],
                                 start=True, stop=False)
                du = T(sp, [128, Q], F16, "du", 2, f"du{tq}_{i}")
                nc.vector.tensor_mul(du[:], deltas[i][:], uconv[i][:, sl])
                dus.append(du)
                # deferred yfin of the previous block so the Vector engine
                # never waits on the Pool/PE tail of iteration i-1
                if pend:
                    pi, pyp = pend.pop()
                    yq = T(sp, [128, Q], F16, "du", 2, f"yq{tq}_{pi}")
                    nc.scalar.copy(out=yq[:], in_=pyp[:])
                    nc.vector.tensor_mul(yfin[pi][:, sl], yq[:],
                                         silz[pi][:, sl])
                for h in range(NH):
                    da = T(sp, [128, WB], F16, "DA", 2, f"da{tq}_{i}_{h}")
                    da3 = r3(da)
                    if tq == 0 and i == 0:
                        # boundary zeros persist across buffer reuse: the
                        # exps only ever write the 513-stride interiors, so
                        # zeroing the first two allocations (both bufs) is
                        # enough for every later tile in the cycle
                        nc.vector.tensor_copy(out=da3[:, :, 0:1],
                                              in_=zcol[:])
                    for nl in range(NPH):
                        n = h * NPH + nl
                        nc.scalar.activation(
                            out=da3[:, nl:nl + 1, 1:Q + 1],
                            in_=deltas[i][:], func=AF.Exp,
                            scale=An[i][:, n:n + 1])
                    dbu = T(sp, [128, WB], F16, "DBU", 1, f"dbu{tq}_{i}_{h}")
                    dbu3 = r3(dbu)
                    if tq == 0:
                        # single buffer: zeros written once keep holding
                        if i == 0 and h == 0:
                            nc.vector.tensor_copy(out=dbu3[:, :, 0:1],
                                                  in_=zcol[:])
                    else:
                        nc.vector.tensor_copy(
                            out=dbu3[:, :, 0:1],
                            in_=states[i][:, h * NPH:(h + 1) * NPH])
                    du_b = dus[i][:].unsqueeze(1).broadcast_to(
                        [128, NPH, Q])
                    nc.vector.tensor_tensor(out=dbu3[:, :, 1:Q + 1],
                                            in0=du_b,
                                            in1=r3(ball[h])[:, :, 1:Q + 1],
                                            op=OP.mult)
                    ht = T(sp, [128, WB], F16, "H", 2, f"h{tq}_{i}_{h}")
                    nc.vector.tensor_tensor_scan(ht[:], da[:], dbu[:], 0.0,
                                                 OP.mult, OP.add)
                    h3 = r3(ht)
                    if tq < TC - 1:
                        nc.vector.tensor_copy(
                            out=states[i][:, h * NPH:(h + 1) * NPH],
                            in_=h3[:, :, Q:Q + 1])
                    hc = T(sp, [128, NPH * Q], F16, "HC", 1,
                           f"hc{tq}_{i}_{h}")
                    hc3 = hc[:].rearrange("p (n q) -> p n q", n=NPH)
                    nc.vector.tensor_tensor(out=hc3, in0=h3[:, :, 1:Q + 1],
                                            in1=r3(call_[h])[:, :, 1:Q + 1],
                                            op=OP.mult)
                    for nl in range(NPH):
                        nc.tensor.matmul(yp[:], ident[:],
                                         hc[:, nl * Q:(nl + 1) * Q],
                                         start=False,
                                         stop=(h == NH - 1 and nl == NPH - 1))
                pend.append((i, yp))
            pi, pyp = pend.pop()
            yq = T(sp, [128, Q], F16, "du", 2, f"yq{tq}_{pi}")
            nc.scalar.copy(out=yq[:], in_=pyp[:])
            nc.vector.tensor_mul(yfin[pi][:, sl], yq[:], silz[pi][:, sl])

        # ---- out_proj + final proj, emitted per t-chunk ----
        yo = [T(hp, [128, L], F16, "mid", 8, f"yo{cb}") for cb in range(NCB)]
        pw = []

        def tail_chunk(t):
            # out_proj: outw streamed through the (now free) craw-tag bufs
            # in two waves of 4 per output block
            for cb in range(NCB):
                ps = T(pp, [128, Q], F32, "dtm", 2, f"ps_o{cb}_{t}")
                for wv in range(2):
                    oww = []
                    for j in range(4):
                        i = wv * 4 + j
                        w_ = T(st, [128, C], F16, "craw", 4,
                               f"outw{t}_{cb}_{i}")
                        dma(out=w_[:], in_=d_outw[i])
                        oww.append(w_)
                    for j in range(4):
                        i = wv * 4 + j
                        nc.tensor.matmul(
                            ps[:], oww[j][:, cb * 128:(cb + 1) * 128],
                            yfin[i][:, t * Q:(t + 1) * Q],
                            start=(i == 0), stop=(i == NDB - 1))
                nc.vector.tensor_copy(out=yo[cb][:, t * Q:(t + 1) * Q],
                                      in_=ps[:])
            if not pw:
                for cib in range(NCB):
                    w_ = T(hp, [128, C], F16, f"uconv{cib}", 1, f"pw{cib}")
                    dma(out=w_[:], in_=d_pw[cib])
                    pw.append(w_)
            for cb in range(NCB):
                ps = T(pp, [128, Q], F32, "mm", 2, f"ps_p{cb}_{t}")
                for cib in range(NCB):
                    nc.tensor.matmul(ps[:],
                                     pw[cib][:, cb * 128:(cb + 1) * 128],
                                     yo[cib][:, t * Q:(t + 1) * Q],
                                     start=(cib == 0), stop=(cib == NCB - 1))
                ot = T(hp, [128, Q], F32, "osb", 1, f"osb{cb}_{t}")
                nc.scalar.copy(out=ot[:], in_=ps[:])
                dma(out=d_part[cb * 128:(cb + 1) * 128, t * Q:(t + 1) * Q],
                    in_=ot[:])

        with tc.high_priority(offset=-1000000):
            tail_chunk(0)
        tail_chunk(1)

    nc.compile()
    return nc


_cache = {}


def _prep_core_inputs(inputs, core):
    b = core >> 1
    rev = (core & 1) == 1
    p = "b_" if rev else "f_"
    f16 = np.float16
    f32 = np.float32

    toks = np.asarray(inputs["x"][b]).astype(np.int64)
    if rev:
        toks = toks[::-1]
    embf = np.asarray(inputs["emb"]).astype(f16)
    h0 = np.ascontiguousarray(embf[toks].T)

    key = ("wts", p)
    if key not in _cache:

        cw = np.asarray(inputs["conv_w"]).astype(f32)  # [D, cout, cin, K]
        if rev:
            cw = cw[:, :, :, ::-1]
        convw = np.empty((DEPTH, NCB, 128, K * NCB, 128), f16)
        for l in range(DEPTH):
            for cib in range(NCB):
                for k in range(K):
                    for cob in range(NCB):
                        blk = cw[l, cob * 128:(cob + 1) * 128,
                                 cib * 128:(cib + 1) * 128, k]
                        convw[l, cib, :, k * NCB + cob, :] = blk.T.astype(f16)
        cpar = np.zeros((DEPTH, 128, 12), f32)
        for l in range(DEPTH):
            for cob in range(NCB):
                cs = slice(cob * 128, (cob + 1) * 128)
                cpar[l, :, 0 * NCB + cob] = inputs["conv_b"][l][cs]
                cpar[l, :, 1 * NCB + cob] = inputs["ln_g"][l][cs]
                cpar[l, :, 2 * NCB + cob] = inputs["ln_b"][l][cs]

        in_w = np.asarray(inputs[p + "in_w"]).astype(f32)  # [2*DI, C]
        inw = np.empty((NCB, 128, 2 * DI), f16)
        for cib in range(NCB):
            inw[cib] = in_w[:, cib * 128:(cib + 1) * 128].T.astype(f16)

        mconv = np.asarray(inputs[p + "conv_w"]).astype(f32)  # [DI, 4]
        mcw = np.zeros((NDB, 128, DCONV * 128), f16)
        dd = np.arange(128)
        for i in range(NDB):
            for k in range(DCONV):
                mcw[i, dd, k * 128 + dd] = mconv[i * 128:(i + 1) * 128, k]

        mpar = np.zeros((128, 16), f32)
        for i in range(NDB):
            mpar[:, i] = inputs[p + "conv_b"][i * 128:(i + 1) * 128]
            mpar[:, 8 + i] = inputs[p + "dt_b"][i * 128:(i + 1) * 128]

        x_w = np.asarray(inputs[p + "x_w"]).astype(f32)  # [64, DI]
        xw = np.empty((NDB, 128, DTR + 2 * N), f16)
        for i in range(NDB):
            xw[i] = x_w[:, i * 128:(i + 1) * 128].T.astype(f16)

        dt_w = np.asarray(inputs[p + "dt_w"]).astype(f32)  # [DI, DTR]
        dtw = np.empty((NDB, DTR, 128), f16)
        for i in range(NDB):
            dtw[i] = dt_w[i * 128:(i + 1) * 128, :].T.astype(f16)

        An = (-np.exp(np.asarray(inputs[p + "A_log"]).astype(f32))
              ).reshape(NDB, 128, N).astype(f32)

        Dv = np.asarray(inputs[p + "D"]).astype(f32)
        Dd = np.zeros((NDB, 128, 128), f16)
        for i in range(NDB):
            Dd[i, dd, dd] = Dv[i * 128:(i + 1) * 128]

        out_w = np.asarray(inputs[p + "out_w"]).astype(f32)  # [C, DI]
        outw = np.empty((NDB, 128, C), f16)
        for i in range(NDB):
            outw[i] = out_w[:, i * 128:(i + 1) * 128].T.astype(f16)

        proj_w = np.asarray(inputs["proj_w"]).astype(f32)  # [C, 2C]
        half = proj_w[:, C:] if rev else proj_w[:, :C]
        pw = np.empty((NCB, 128, C), f16)
        for cib in range(NCB):
            pw[cib] = half[:, cib * 128:(cib + 1) * 128].T.astype(f16)

        _cache[key] = dict(
            convw=convw, cpar=cpar, inw=inw, mcw=mcw, mpar=mpar,
            xw=xw, dtw=dtw, An=An, Dd=Dd, outw=outw, pw=pw,
            ident=np.eye(128, dtype=f16))
    m = dict(_cache[key])
    m["h0"] = h0
    return m


def kernel(**inputs):
    if "nc" not in _cache:
        _cache["nc"] = build_program()
    nc = _cache["nc"]
    # weights are cached per direction for repeat calls; invalidate so a new
    # inputs dict is always re-prepared
    for k in [k for k in _cache if k != "nc"]:
        del _cache[k]
    in_maps = [_prep_core_inputs(inputs, c) for c in range(8)]
    res = run_bass_kernel_spmd(nc, in_maps, list(range(8)))
    parts = [r["part"] for r in res.results]
    proj_b = np.asarray(inputs["proj_b"]).astype(np.float32)
    out = np.empty((B, L, C), np.float32)
    for b in range(B):
        # note: the reference concatenates bo still in reversed time order
        comb = parts[2 * b] + parts[2 * b + 1]
        out[b] = comb.T + proj_b[None, :]
    m = np.asarray(inputs["m"])
    out = np.where(m[:, :, None], 0.0, out).astype(np.float32)
    return out
